# revision 33
# baseline (speedup 1.0000x reference)
"""Trainium2 Bass kernel for nn_NodeEdgeCrossAttention.

Strategy (dst-sharded, zero-collective):
  - Host sorts edges by destination node, assigns nodes to 8 cores with
    balanced padded-edge counts, and packs each node's edge run (padded to a
    multiple of 32) into 512-column chunks using a slot pattern shared by all
    cores (SPMD requires one program).
  - Scores fold Wq/Wk into per-node M matrices (score = M[dst] . k_raw) so no
    k-projection or q-gather is needed.  bk cancels by softmax shift
    invariance; bv folds through Wo into bo because sum(attn) == 1.
  - Scores are edge-major: per 128-col tile ONE matmul with the kc tile as
    the (FWL-fast, full-width) stationary operand and the 16 M columns of the
    tile's <=4 slot pieces as moving.  A host-packed mask zeroes the
    off-piece garbage after exp, and a jj-reduction yields exE [128, (t,h)].
  - v is packed EDGE-major with a ones column; T = S (x) exE is built by DVE
    and used as the segment-matmul stationary against raw v, accumulating
    raw per-(head,slot) sums AND softmax denominators in one PSUM tile.
    Wv never touches per-edge data: out = (Braw_h/den) @ (Wv_h @ Wo) summed
    over heads in the final stage (exact by linearity).
  - Segment matmuls for chunk ci are emitted one iteration late so the PE
    never stalls on the scalar->vector chain.  Park groups of 3 chunks drain
    to a DRAM scratch by DMA.
  - Numerics: fp16 k/v/T (exp scaled by 2^-6 to fit f16 range), bf16 only
    for the pre-mask exp values, fp32 accumulation.
"""

import numpy as np

N, E, DIM, HEADS = 10000, 640000, 128, 4
DH = DIM // HEADS
NCORES = 8
CHUNK = 512
TILE = 128
SCALE = DH ** -0.5
CW = DIM + HEADS     # 132: park row width (braw | denom @ col 128)
VW = DIM + 4         # 132: per-tile v_em width (v | ones | pad)
JJ = 4               # slot-piece grid per tile
GPC = 3              # chunks per PSUM park group
EXB = -4.158883083359672   # exp bias: -6*ln(2), cancels in normalization


class Plan:
    pass


def _make_plan(dst):
    """Pack nodes at exact-degree granularity into a chunk/slot layout
    shared across all 8 cores.  No alignment padding: slots occupy
    arbitrary contiguous column runs, split at 128-col tile boundaries
    into <= JJ pieces per tile."""
    deg = np.bincount(dst, minlength=N)
    if deg.max() > 128:
        raise NotImplementedError(f"max degree {deg.max()} > 128 needs node splitting")

    order = np.argsort(-deg, kind="stable")
    order = order[deg[order] > 0]
    loads = np.zeros(NCORES, np.int64)
    core_nodes = [[] for _ in range(NCORES)]
    for n in order:
        c = int(loads.argmin())
        core_nodes[c].append(int(n))
        loads[c] += deg[n]

    # Shared slot pattern = elementwise max over cores' (desc) deg seqs.
    L = max(len(cn) for cn in core_nodes)
    pat = np.zeros(L, np.int64)
    for cn in core_nodes:
        r = deg[np.array(cn, np.int64)]
        pat[: len(r)] = np.maximum(pat[: len(r)], r)

    SLMAX = 16
    slots = []           # {R, chunk, col0, pi}
    chunks = []          # {slots: [...], tilecnt: [...]}

    def new_chunk():
        chunks.append({"slots": [], "tilecnt": [0, 0, 0, 0]})

    new_chunk()
    ptr = 0
    for pi in range(L):
        R = int(pat[pi])
        while True:
            ch = chunks[-1]
            if ptr + R <= CHUNK and len(ch["slots"]) < SLMAX:
                t0 = ptr // TILE
                t1 = (ptr + R - 1) // TILE
                if all(ch["tilecnt"][t] < JJ for t in range(t0, t1 + 1)):
                    ch["slots"].append(len(slots))
                    slots.append({"R": R, "chunk": len(chunks) - 1,
                                  "col0": ptr, "pi": pi})
                    for t in range(t0, t1 + 1):
                        ch["tilecnt"][t] += 1
                    ptr += R
                    break
            # advance to next tile start (or next chunk)
            nxt = (ptr // TILE + 1) * TILE
            if nxt >= CHUNK or len(ch["slots"]) >= SLMAX:
                new_chunk()
                ptr = 0
            else:
                ptr = nxt
    if not chunks[-1]["slots"]:
        chunks.pop()

    max_ns = 0
    for ch in chunks:
        ch["ns"] = len(ch["slots"])
        max_ns = max(max_ns, ch["ns"])

    # Per-chunk tile pieces: (t, jj, r0, len, slot_j).
    for ci, ch in enumerate(chunks):
        pieces = []
        nxt = [0, 0, 0, 0]
        for j, sidx in enumerate(ch["slots"]):
            s = slots[sidx]
            lo = s["col0"]
            end = s["col0"] + s["R"]
            while lo < end:
                t = lo // TILE
                hi = min(end, (t + 1) * TILE)
                pieces.append({"t": t, "jj": nxt[t], "r0": lo - t * TILE,
                               "len": hi - lo, "j": j, "sidx": sidx,
                               "off": lo - s["col0"]})
                nxt[t] += 1
                lo = hi
        assert max(nxt) <= JJ
        ch["pieces"] = pieces

    p = Plan()
    p.sl = max_ns                                    # slot positions per chunk
    p.kvw = CHUNK + 4 * VW + 4 * p.sl
    p.deg = deg
    p.core_nodes = core_nodes
    p.slots = slots
    p.chunks = chunks
    p.nchunks = len(chunks)
    p.cols = p.nchunks * CHUNK
    p.nslot = p.nchunks * p.sl                       # chunk-slot space
    p.nsp = ((p.nslot + CHUNK - 1) // CHUNK) * CHUNK     # 512-padded
    p.ng2 = p.nchunks * 4 * JJ                       # tile-major piece grid
    p.nsp2 = ((p.ng2 + CHUNK - 1) // CHUNK) * CHUNK
    p.nr2 = p.nchunks * HEADS * p.sl                 # park rows total
    return p


def _pack_core_inputs(plan, c, k_edges, v_edges, q_nodes, edges_of):
    """Per-core fused kvs [128, nchunks*KVW] f16, qT2 [128, nsp2] f16, qslot."""
    cols = plan.cols
    SL = plan.sl
    edge_order = np.full(cols, -1, np.int64)
    qslot = np.full(plan.nslot, -1, np.int64)    # chunk-slot -> node (output)
    qslot2 = np.full(plan.ng2, -1, np.int64)     # (ci,t,jj) piece -> node
    cn = plan.core_nodes[c]
    for ci, ch in enumerate(plan.chunks):
        for j, sidx in enumerate(ch["slots"]):
            s = plan.slots[sidx]
            if s["pi"] < 0 or s["pi"] >= len(cn):
                continue
            node = cn[s["pi"]]
            d = plan.deg[node]
            g0 = ci * CHUNK + s["col0"]
            edge_order[g0: g0 + d] = edges_of[node]
            qslot[ci * SL + j] = node
        for pc in ch["pieces"]:
            s = plan.slots[pc["sidx"]]
            if s["pi"] < 0 or s["pi"] >= len(cn):
                continue
            qslot2[(ci * 4 + pc["t"]) * JJ + pc["jj"]] = cn[s["pi"]]

    valid = edge_order >= 0
    idx = np.where(valid, edge_order, 0)
    kT = np.where(valid[:, None], k_edges[idx], 0.0).astype(np.float16).T
    vE = np.where(valid[:, None], v_edges[idx], 0.0).astype(np.float16)

    # S2: [128, nchunks, 4*SL] one-hot (tile-row, chunk-slot), f16
    S = np.zeros((TILE, plan.nchunks, 4 * SL), np.float16)
    # ST: [32, nchunks*128] complement piece indicator (rows (t,jj)), f16
    ST = np.ones((32, plan.nchunks, TILE), np.float16)
    ST[4 * JJ:] = 0.0
    # ind: [128, nchunks, 4] real-edge indicator (denominator ones-col)
    ind = np.zeros((TILE, plan.nchunks, 4), np.float16)
    for ci, ch in enumerate(plan.chunks):
        for pc in ch["pieces"]:
            s = plan.slots[pc["sidx"]]
            t, r0, ln = pc["t"], pc["r0"], pc["len"]
            ST[t * JJ + pc["jj"], ci, r0:r0 + ln] = 0.0
            if s["pi"] < 0 or s["pi"] >= len(cn):
                continue
            # truncate to the node's actual degree (pattern R may exceed it)
            vln = min(max(plan.deg[cn[s["pi"]]] - pc["off"], 0), ln)
            S[r0:r0 + vln, ci, t * SL + pc["j"]] = 1.0
            ind[r0:r0 + vln, ci, t] = 1.0

    KVW = plan.kvw
    kvs = np.zeros((TILE, plan.nchunks, KVW), np.float16)
    kvs[:, :, 0:CHUNK] = kT.reshape(TILE, plan.nchunks, CHUNK)
    vem = kvs[:, :, CHUNK:CHUNK + 4 * VW].reshape(TILE, plan.nchunks, 4, VW)
    vem[:, :, :, 0:DIM] = vE.reshape(plan.nchunks, 4, TILE, DIM).transpose(2, 0, 1, 3)
    vem[:, :, :, DIM] = ind
    kvs[:, :, CHUNK + 4 * VW:KVW] = S
    kvs = np.ascontiguousarray(kvs.reshape(TILE, plan.nchunks * KVW))
    ST = np.ascontiguousarray(ST.reshape(32, plan.nchunks * TILE))

    qvalid = qslot2 >= 0
    qidx = np.where(qvalid, qslot2, 0)
    qT2 = np.zeros((DIM, plan.nsp2), np.float16)
    qT2[:, : plan.ng2] = np.where(qvalid[:, None], q_nodes[qidx], 0.0
                                  ).astype(np.float16).T
    return kvs, ST, qT2, qslot


# ---------------------------------------------------------------------------
# Device kernel emission
# ---------------------------------------------------------------------------

def _build_module(plan):
    import concourse.bacc as bacc
    import concourse.mybir as mybir
    import concourse.tile as tile
    from contextlib import ExitStack

    f16 = mybir.dt.float16
    bf = mybir.dt.bfloat16
    f32 = mybir.dt.float32
    NSP = plan.nsp
    NSP2 = plan.nsp2
    SL = plan.sl
    KVW = plan.kvw
    PR = HEADS * SL              # park rows per chunk (32)
    NR2 = plan.nr2               # total park rows
    NB2 = (NR2 + TILE - 1) // TILE   # 128-row blocks in final readback

    nc = bacc.Bacc("TRN2", debug=False, num_devices=NCORES)

    kvs_d = nc.dram_tensor("kvs", [TILE, plan.nchunks * KVW], f16,
                           kind="ExternalInput")
    ST_d = nc.dram_tensor("ST", [32, plan.nchunks * TILE], f16,
                          kind="ExternalInput")
    V4_d = nc.dram_tensor("V4", [32, 4 * JJ * HEADS], f16,
                          kind="ExternalInput")
    qT_d = nc.dram_tensor("qT", [DIM, NSP2], f16, kind="ExternalInput")
    Wq_d = nc.dram_tensor("Wq", [DIM, DIM], f16, kind="ExternalInput")
    WkH_d = nc.dram_tensor("WkH", [DIM, HEADS * DIM], f16, kind="ExternalInput")
    WVO_d = nc.dram_tensor("WVO", [DIM, HEADS * DIM], f16, kind="ExternalInput")
    ID_d = nc.dram_tensor("ID", [DIM, DIM], f32, kind="ExternalInput")
    bq_d = nc.dram_tensor("bq", [DIM, 1], f32, kind="ExternalInput")
    exb_d = nc.dram_tensor("exb", [DIM, 1], f32, kind="ExternalInput")
    bo_d = nc.dram_tensor("bo", [DIM, 1], f32, kind="ExternalInput")
    accD = nc.dram_tensor("accD", [NR2, CW], bf, kind="Internal")
    outT_d = nc.dram_tensor("outT", [DIM, NSP], f32, kind="ExternalOutput")

    Exp = mybir.ActivationFunctionType.Exp
    Ident = mybir.ActivationFunctionType.Identity
    mult = mybir.AluOpType.mult
    amax = mybir.AluOpType.max
    addop = mybir.AluOpType.add
    AxX = mybir.AxisListType.X

    with ExitStack() as ctx:
        tc = ctx.enter_context(tile.TileContext(nc))
        cp = ctx.enter_context(tc.tile_pool(name="const", bufs=1))
        sp = ctx.enter_context(tc.tile_pool(name="persist", bufs=1))
        iop = ctx.enter_context(tc.tile_pool(name="io", bufs=8))
        xp = ctx.enter_context(tc.tile_pool(name="work", bufs=6))
        pp = ctx.enter_context(tc.tile_pool(name="ps", bufs=2, space="PSUM"))

        def dmac(tile_ap, dram_ap):
            nc.sync.dma_start(out=tile_ap, in_=dram_ap)

        Wq_sb = cp.tile([DIM, DIM], f16); dmac(Wq_sb[:], Wq_d[:, :])
        WkH_sb = cp.tile([DIM, HEADS * DIM], f16); dmac(WkH_sb[:], WkH_d[:, :])
        WVO_sb = cp.tile([DIM, HEADS * DIM], f16); dmac(WVO_sb[:], WVO_d[:, :])
        ID_sb = cp.tile([DIM, DIM], f32); dmac(ID_sb[:], ID_d[:, :])
        V4_sb = cp.tile([32, 4 * JJ * HEADS], f16); dmac(V4_sb[:], V4_d[:, :])
        bq_sb = cp.tile([DIM, 1], f32); dmac(bq_sb[:], bq_d[:, :])
        exb_sb = cp.tile([DIM, 1], f32); dmac(exb_sb[:], exb_d[:, :])
        bo_sb = cp.tile([DIM, 1], f32); dmac(bo_sb[:], bo_d[:, :])
        qT_sb = sp.tile([DIM, NSP2], f16); dmac(qT_sb[:], qT_d[:, :])

        qp_sb = sp.tile([DIM, NSP2], f16)
        M_sb = sp.tile([DIM, 4 * NSP2], f16)

        # ---- Stage A: q projection + bias ----
        for b in range(NSP2 // CHUNK):
            sl = slice(b * CHUNK, (b + 1) * CHUNK)
            qp_ps = pp.tile([DIM, CHUNK], f32, tag="aux")
            nc.tensor.matmul(out=qp_ps[:], lhsT=Wq_sb[:], rhs=qT_sb[:, sl],
                             start=True, stop=True)
            nc.scalar.activation(out=qp_sb[:, sl], in_=qp_ps[:],
                                 func=Ident, bias=bq_sb[:, 0:1])

        # ---- Stage A: M matrices, (w,h)-interleaved, emitted JIT ----
        M_wh = M_sb[:].rearrange("p (w h) -> p h w", h=HEADS)
        NG = NSP2 // CHUNK

        def emit_mgroup(b):
            sl = slice(b * CHUNK, (b + 1) * CHUNK)
            for h in range(HEADS):
                M_ps = pp.tile([DIM, CHUNK], f32, tag="aux", name=f"M_ps{b}_{h}")
                nc.tensor.matmul(out=M_ps[:],
                                 lhsT=WkH_sb[:, h * DIM:(h + 1) * DIM],
                                 rhs=qp_sb[:, sl], start=True, stop=True)
                nc.scalar.copy(out=M_wh[:, h, sl], in_=M_ps[:])

        emit_mgroup(0)
        emit_mgroup(1)
        next_g = 2

        # ---- Steady state: chunk PAIRS; seg matmuls lag one pair behind ----
        npairs = (plan.nchunks + 1) // 2
        park = None
        pend = {}
        for pi2 in range(npairs + 2):
            while next_g < NG and pi2 >= 4 * next_g - 6:
                emit_mgroup(next_g)
                next_g += 1
            if pi2 < npairs:
                c0 = 2 * pi2
                pcs = [c for c in (c0, c0 + 1) if c < plan.nchunks]
                np_ = len(pcs)
                kv2 = iop.tile([TILE, 2 * KVW], f16, tag="kv")
                dmac(kv2[:, 0:np_ * KVW],
                     kvs_d[:, c0 * KVW:(c0 + np_) * KVW])
                st2 = iop.tile([32, 2 * TILE], f16, tag="st")
                dmac(st2[:, 0:np_ * TILE],
                     ST_d[:, c0 * TILE:(c0 + np_) * TILE])

                ex_ps = pp.tile([TILE, 2 * 4 * JJ * HEADS], f32, tag="score")
                for c2, ci in enumerate(pcs):
                    nc.tensor.matmul(
                        out=ex_ps[:, c2 * 64:(c2 + 1) * 64],
                        lhsT=st2[:, c2 * TILE:(c2 + 1) * TILE],
                        rhs=V4_sb[:],
                        start=True, stop=True)
                    for t in range(4):
                        m0 = 4 * ((ci * 4 + t) * JJ)
                        nc.tensor.matmul(
                            out=ex_ps[:, c2 * 64 + t * 16:c2 * 64 + t * 16 + 16],
                            lhsT=kv2[:, c2 * KVW + t * TILE:
                                     c2 * KVW + (t + 1) * TILE],
                            rhs=M_sb[:, m0:m0 + JJ * HEADS],
                            start=False, stop=True)

                exR_sb = xp.tile([TILE, 2 * 64], bf, tag="exR")
                nc.scalar.activation(out=exR_sb[:, 0:np_ * 64],
                                     in_=ex_ps[:, 0:np_ * 64], func=Exp,
                                     bias=exb_sb[:, 0:1])
                T_sb = xp.tile([TILE, 2 * 4 * HEADS * SL], f16, tag="T")
                for c2, ci in enumerate(pcs):
                    exE_sb = xp.tile([TILE, 4 * HEADS], f32, tag="exE")
                    nc.vector.tensor_reduce(
                        out=exE_sb[:].rearrange("p (t h) -> p t h", t=4),
                        in_=exR_sb[:, c2 * 64:(c2 + 1) * 64]
                            .rearrange("p (t q h) -> p t h q", t=4, q=JJ),
                        axis=AxX, op=addop)
                    S2v = kv2[:, c2 * KVW + CHUNK + 4 * VW:
                              c2 * KVW + CHUNK + 4 * VW + 4 * SL]
                    nc.vector.tensor_tensor(
                        out=T_sb[:, c2 * 4 * PR:(c2 + 1) * 4 * PR]
                            .rearrange("p (t h j) -> p t h j", t=4, h=HEADS),
                        in0=S2v.rearrange("p (t j) -> p t j", t=4)
                            .unsqueeze(2).to_broadcast([TILE, 4, HEADS, SL]),
                        in1=exE_sb[:].rearrange("p (t h) -> p t h", t=4)
                            .unsqueeze(-1).to_broadcast([TILE, 4, HEADS, SL]),
                        op=mult)
                pend[pi2] = (kv2, T_sb, pcs)

            if pi2 >= 2:
                kv2j, T_j, pcsj = pend.pop(pi2 - 2)
                for c2, cj in enumerate(pcsj):
                    gi = cj % GPC
                    if gi == 0:
                        park = pp.tile([PR, GPC * CW], f32, tag="park")
                    for t in range(4):
                        nc.tensor.matmul(
                            out=park[:, gi * CW:(gi + 1) * CW],
                            lhsT=T_j[:, c2 * 4 * PR + t * PR:
                                     c2 * 4 * PR + (t + 1) * PR],
                            rhs=kv2j[:, c2 * KVW + CHUNK + t * VW:
                                     c2 * KVW + CHUNK + t * VW + CW],
                            start=(t == 0), stop=(t == 3))
                    if gi == GPC - 1 or cj == plan.nchunks - 1:
                        g0 = (cj // GPC) * GPC
                        used = cj - g0 + 1
                        stage = xp.tile([PR, GPC * CW], bf, tag="stage")
                        nc.vector.tensor_copy(out=stage[:, 0:used * CW],
                                              in_=park[:, 0:used * CW])
                        nc.scalar.dma_start(
                            out=accD[g0 * PR:(cj + 1) * PR, :]
                                .rearrange("(c r) w -> r c w", r=PR),
                            in_=stage[:, 0:used * CW]
                                .rearrange("r (c w) -> r c w", w=CW))

        # ---- Final: readback, normalize, project with Wv@Wo per head ----
        # Two halves so half A's normalize/transpose overlaps half B's DMA.
        accR = sp.tile([TILE, NB2 * CW], bf)
        nc.gpsimd.memset(accR[:], 0.0)
        rden_sb = sp.tile([TILE, NB2], f32)
        nB_sb = sp.tile([TILE, NB2 * DIM], f32)
        nBT_sb = sp.tile([TILE, NB2 * TILE], f16)
        qs = [0, NB2 // 4, NB2 // 2, 3 * NB2 // 4, NB2]
        for b0, b1 in zip(qs, qs[1:]):
            full = min(b1 * TILE, NR2) // TILE - b0
            if full > 0:
                dmac(accR[:, b0 * CW:(b0 + full) * CW]
                     .rearrange("p (b w) -> p b w", w=CW),
                     accD[b0 * TILE:(b0 + full) * TILE, :]
                     .rearrange("(b p) w -> p b w", p=TILE))
            bf_ = b0 + full
            tail = NR2 - bf_ * TILE
            if 0 < tail < TILE and bf_ < b1:
                dmac(accR[0:tail, bf_ * CW:(bf_ + 1) * CW],
                     accD[bf_ * TILE:NR2, :])
            nc.vector.tensor_scalar(
                out=rden_sb[:, b0:b1].unsqueeze(-1),
                in0=accR[:, b0 * CW:b1 * CW]
                    .rearrange("p (b w) -> p b w", w=CW)[:, :, DIM:DIM + 1],
                scalar1=1e-30, scalar2=None, op0=amax)
            nc.vector.reciprocal(out=rden_sb[:, b0:b1], in_=rden_sb[:, b0:b1])
            nc.vector.tensor_tensor(
                out=nB_sb[:, b0 * DIM:b1 * DIM]
                    .rearrange("p (b w) -> p b w", w=DIM),
                in0=accR[:, b0 * CW:b1 * CW]
                    .rearrange("p (b w) -> p b w", w=CW)[:, :, 0:DIM],
                in1=rden_sb[:, b0:b1].unsqueeze(-1)
                    .to_broadcast([TILE, b1 - b0, DIM]),
                op=mult)
            for b in range(b0, b1):
                tp_ps = pp.tile([DIM, TILE], f32, tag="tp")
                nc.tensor.transpose(out=tp_ps[:],
                                    in_=nB_sb[:, b * DIM:(b + 1) * DIM],
                                    identity=ID_sb[:])
                nc.vector.tensor_copy(out=nBT_sb[:, b * TILE:(b + 1) * TILE],
                                      in_=tp_ps[:])

        # nBT cols are park rows: (ci, h, j).  For each ci-group and head,
        # a strided rhs selects that head's SL slot columns.
        nBT_r = nBT_sb[:, 0:NR2].rearrange("p (c h j) -> p c h j", h=HEADS, j=SL)
        CIG = CHUNK // SL        # chunks per output group (64)
        for gstart in range(0, plan.nchunks, CIG):
            gend = min(gstart + CIG, plan.nchunks)
            ncol = (gend - gstart) * SL
            out_ps = pp.tile([DIM, CHUNK], f32, tag="aux")
            for h in range(HEADS):
                nc.tensor.matmul(
                    out=out_ps[:, 0:ncol],
                    lhsT=WVO_sb[:, h * DIM:(h + 1) * DIM],
                    rhs=nBT_r[:, gstart:gend, h, :],
                    start=(h == 0), stop=(h == HEADS - 1))
            osb = xp.tile([DIM, CHUNK], f32, tag="osb")
            nc.scalar.activation(out=osb[:, 0:ncol], in_=out_ps[:, 0:ncol],
                                 func=Ident, bias=bo_sb[:, 0:1])
            dmac(outT_d[:, gstart * SL:gstart * SL + ncol], osb[:, 0:ncol])

    nc.compile()
    return nc


# ---------------------------------------------------------------------------
# Entry point
# ---------------------------------------------------------------------------

def _make_v4():
    BIG = 50.0
    V4 = np.zeros((32, 4 * JJ * HEADS), np.float16)
    for z in range(4 * JJ):
        V4[z, z * HEADS:(z + 1) * HEADS] = -BIG
    return V4


def _prepare(inputs):
    q_nodes = np.asarray(inputs["q_nodes"], np.float32)
    k_edges = np.asarray(inputs["k_edges"], np.float32)
    v_edges = np.asarray(inputs["v_edges"], np.float32)
    Wq = np.asarray(inputs["Wq"], np.float32)
    bq = np.asarray(inputs["bq"], np.float32)
    Wk = np.asarray(inputs["Wk"], np.float32)
    Wv = np.asarray(inputs["Wv"], np.float32)
    bv = np.asarray(inputs["bv"], np.float32)
    Wo = np.asarray(inputs["Wo"], np.float32)
    bo = np.asarray(inputs["bo"], np.float32)
    dst = np.asarray(inputs["edge_index"])[0].astype(np.int64)

    plan = _make_plan(dst)

    eorder = np.argsort(dst, kind="stable")
    starts = np.zeros(N + 1, np.int64)
    np.cumsum(np.bincount(dst, minlength=N), out=starts[1:])
    edges_of = [eorder[starts[n]: starts[n + 1]] for n in range(N)]

    # WVO[:, h-block] = Wv[:, 32h:32h+32] @ Wo[32h:32h+32, :]
    WVO = np.concatenate(
        [Wv[:, h * DH:(h + 1) * DH] @ Wo[h * DH:(h + 1) * DH, :]
         for h in range(HEADS)], axis=1)

    consts = {
        "Wq": Wq.astype(np.float16),
        "WkH": np.concatenate(
            [np.where((np.arange(DIM)[:, None] // DH == h),
                      (Wk * SCALE).T, 0.0) for h in range(HEADS)],
            axis=1).astype(np.float16),
        "WVO": WVO.astype(np.float16),
        "ID": np.eye(DIM, dtype=np.float32),
        "V4": _make_v4(),
        "bq": bq.reshape(DIM, 1).astype(np.float32),
        "exb": np.full((DIM, 1), EXB, np.float32),
        # sum(attn)==1 folds bv through Wo: out = (segv/den)@Wo + (bv@Wo + bo)
        "bo": (bv @ Wo + bo).reshape(DIM, 1).astype(np.float32),
    }
    return plan, dst, edges_of, consts, q_nodes, k_edges, v_edges, bo


def kernel(**inputs):
    from concourse.bass_utils import run_bass_kernel_spmd

    (plan, dst, edges_of, consts, q_nodes, k_edges, v_edges, bo) = _prepare(inputs)

    nc = _build_module(plan)

    in_maps = []
    slot_maps = []
    for c in range(NCORES):
        kvs, ST, qT2, qslot = _pack_core_inputs(plan, c, k_edges, v_edges,
                                                q_nodes, edges_of)
        m = {"kvs": kvs, "ST": ST, "qT": qT2}
        m.update(consts)
        in_maps.append(m)
        slot_maps.append(qslot)

    res = run_bass_kernel_spmd(nc, in_maps, core_ids=list(range(NCORES)))
    global LAST_RESULTS
    LAST_RESULTS = res

    out = np.zeros((N, DIM), np.float32)
    for c in range(NCORES):
        outT = res.results[c]["outT"]          # [DIM, nsp]
        qslot = slot_maps[c]
        valid = qslot >= 0
        out[qslot[valid]] = outT[:, : plan.nslot].T[valid]
    deg0 = plan.deg == 0
    if deg0.any():
        out[deg0] = bo
    return out


# revision 34
# speedup vs baseline: 1.0068x; 1.0068x over previous
"""Trainium2 Bass kernel for nn_NodeEdgeCrossAttention.

Strategy (dst-sharded, zero-collective):
  - Host sorts edges by destination node, assigns nodes to 8 cores with
    balanced padded-edge counts, and packs each node's edge run (padded to a
    multiple of 32) into 512-column chunks using a slot pattern shared by all
    cores (SPMD requires one program).
  - Scores fold Wq/Wk into per-node M matrices (score = M[dst] . k_raw) so no
    k-projection or q-gather is needed.  bk cancels by softmax shift
    invariance; bv folds through Wo into bo because sum(attn) == 1.
  - Scores are edge-major: per 128-col tile ONE matmul with the kc tile as
    the (FWL-fast, full-width) stationary operand and the 16 M columns of the
    tile's <=4 slot pieces as moving.  A host-packed mask zeroes the
    off-piece garbage after exp, and a jj-reduction yields exE [128, (t,h)].
  - v is packed EDGE-major with a ones column; T = S (x) exE is built by DVE
    and used as the segment-matmul stationary against raw v, accumulating
    raw per-(head,slot) sums AND softmax denominators in one PSUM tile.
    Wv never touches per-edge data: out = (Braw_h/den) @ (Wv_h @ Wo) summed
    over heads in the final stage (exact by linearity).
  - Segment matmuls for chunk ci are emitted one iteration late so the PE
    never stalls on the scalar->vector chain.  Park groups of 3 chunks drain
    to a DRAM scratch by DMA.
  - Numerics: fp16 k/v/T (exp scaled by 2^-6 to fit f16 range), bf16 only
    for the pre-mask exp values, fp32 accumulation.
"""

import numpy as np

N, E, DIM, HEADS = 10000, 640000, 128, 4
DH = DIM // HEADS
NCORES = 8
CHUNK = 512
TILE = 128
SCALE = DH ** -0.5
CW = DIM + HEADS     # 132: park row width (braw | denom @ col 128)
VW = DIM + 4         # 132: per-tile v_em width (v | ones | pad)
JJ = 4               # slot-piece grid per tile
GPC = 3              # chunks per PSUM park group
EXB = -4.158883083359672   # exp bias: -6*ln(2), cancels in normalization


class Plan:
    pass


def _make_plan(dst):
    """Pack nodes at exact-degree granularity into a chunk/slot layout
    shared across all 8 cores.  No alignment padding: slots occupy
    arbitrary contiguous column runs, split at 128-col tile boundaries
    into <= JJ pieces per tile."""
    deg = np.bincount(dst, minlength=N)
    if deg.max() > 128:
        raise NotImplementedError(f"max degree {deg.max()} > 128 needs node splitting")

    order = np.argsort(-deg, kind="stable")
    order = order[deg[order] > 0]
    loads = np.zeros(NCORES, np.int64)
    core_nodes = [[] for _ in range(NCORES)]
    for n in order:
        c = int(loads.argmin())
        core_nodes[c].append(int(n))
        loads[c] += deg[n]

    # Shared slot pattern = elementwise max over cores' (desc) deg seqs.
    L = max(len(cn) for cn in core_nodes)
    pat = np.zeros(L, np.int64)
    for cn in core_nodes:
        r = deg[np.array(cn, np.int64)]
        pat[: len(r)] = np.maximum(pat[: len(r)], r)

    SLMAX = 16
    slots = []           # {R, chunk, col0, pi}
    chunks = []          # {slots: [...], tilecnt: [...]}

    def new_chunk():
        chunks.append({"slots": [], "tilecnt": [0, 0, 0, 0]})

    new_chunk()
    ptr = 0
    for pi in range(L):
        R = int(pat[pi])
        while True:
            ch = chunks[-1]
            if ptr + R <= CHUNK and len(ch["slots"]) < SLMAX:
                t0 = ptr // TILE
                t1 = (ptr + R - 1) // TILE
                if all(ch["tilecnt"][t] < JJ for t in range(t0, t1 + 1)):
                    ch["slots"].append(len(slots))
                    slots.append({"R": R, "chunk": len(chunks) - 1,
                                  "col0": ptr, "pi": pi})
                    for t in range(t0, t1 + 1):
                        ch["tilecnt"][t] += 1
                    ptr += R
                    break
            # advance to next tile start (or next chunk)
            nxt = (ptr // TILE + 1) * TILE
            if nxt >= CHUNK or len(ch["slots"]) >= SLMAX:
                new_chunk()
                ptr = 0
            else:
                ptr = nxt
    if not chunks[-1]["slots"]:
        chunks.pop()

    max_ns = 0
    for ch in chunks:
        ch["ns"] = len(ch["slots"])
        max_ns = max(max_ns, ch["ns"])

    # Per-chunk tile pieces: (t, jj, r0, len, slot_j).
    for ci, ch in enumerate(chunks):
        pieces = []
        nxt = [0, 0, 0, 0]
        for j, sidx in enumerate(ch["slots"]):
            s = slots[sidx]
            lo = s["col0"]
            end = s["col0"] + s["R"]
            while lo < end:
                t = lo // TILE
                hi = min(end, (t + 1) * TILE)
                pieces.append({"t": t, "jj": nxt[t], "r0": lo - t * TILE,
                               "len": hi - lo, "j": j, "sidx": sidx,
                               "off": lo - s["col0"]})
                nxt[t] += 1
                lo = hi
        assert max(nxt) <= JJ
        ch["pieces"] = pieces

    p = Plan()
    p.sl = max_ns                                    # slot positions per chunk
    p.kvw = CHUNK + 4 * VW + 4 * p.sl
    p.deg = deg
    p.core_nodes = core_nodes
    p.slots = slots
    p.chunks = chunks
    p.nchunks = len(chunks)
    p.cols = p.nchunks * CHUNK
    p.nslot = p.nchunks * p.sl                       # chunk-slot space
    p.nsp = ((p.nslot + CHUNK - 1) // CHUNK) * CHUNK     # 512-padded
    p.ng2 = p.nchunks * 4 * JJ                       # tile-major piece grid
    p.nsp2 = ((p.ng2 + CHUNK - 1) // CHUNK) * CHUNK
    p.nr2 = p.nchunks * HEADS * p.sl                 # park rows total
    return p


def _pack_core_inputs(plan, c, k_edges, v_edges, q_nodes, edges_of):
    """Per-core fused kvs [128, nchunks*KVW] f16, qT2 [128, nsp2] f16, qslot."""
    cols = plan.cols
    SL = plan.sl
    edge_order = np.full(cols, -1, np.int64)
    qslot = np.full(plan.nslot, -1, np.int64)    # chunk-slot -> node (output)
    qslot2 = np.full(plan.ng2, -1, np.int64)     # (ci,t,jj) piece -> node
    cn = plan.core_nodes[c]
    for ci, ch in enumerate(plan.chunks):
        for j, sidx in enumerate(ch["slots"]):
            s = plan.slots[sidx]
            if s["pi"] < 0 or s["pi"] >= len(cn):
                continue
            node = cn[s["pi"]]
            d = plan.deg[node]
            g0 = ci * CHUNK + s["col0"]
            edge_order[g0: g0 + d] = edges_of[node]
            qslot[ci * SL + j] = node
        for pc in ch["pieces"]:
            s = plan.slots[pc["sidx"]]
            if s["pi"] < 0 or s["pi"] >= len(cn):
                continue
            qslot2[(ci * 4 + pc["t"]) * JJ + pc["jj"]] = cn[s["pi"]]

    valid = edge_order >= 0
    idx = np.where(valid, edge_order, 0)
    kT = np.where(valid[:, None], k_edges[idx], 0.0).astype(np.float16).T
    vE = np.where(valid[:, None], v_edges[idx], 0.0).astype(np.float16)

    # S2: [128, nchunks, 4*SL] one-hot (tile-row, chunk-slot), f16
    S = np.zeros((TILE, plan.nchunks, 4 * SL), np.float16)
    # ST: [32, nchunks*128] complement piece indicator (rows (t,jj)), f16
    ST = np.ones((32, plan.nchunks, TILE), np.float16)
    ST[4 * JJ:] = 0.0
    # ind: [128, nchunks, 4] real-edge indicator (denominator ones-col)
    ind = np.zeros((TILE, plan.nchunks, 4), np.float16)
    for ci, ch in enumerate(plan.chunks):
        for pc in ch["pieces"]:
            s = plan.slots[pc["sidx"]]
            t, r0, ln = pc["t"], pc["r0"], pc["len"]
            ST[t * JJ + pc["jj"], ci, r0:r0 + ln] = 0.0
            if s["pi"] < 0 or s["pi"] >= len(cn):
                continue
            # truncate to the node's actual degree (pattern R may exceed it)
            vln = min(max(plan.deg[cn[s["pi"]]] - pc["off"], 0), ln)
            S[r0:r0 + vln, ci, t * SL + pc["j"]] = 1.0
            ind[r0:r0 + vln, ci, t] = 1.0

    KVW = plan.kvw
    kvs = np.zeros((TILE, plan.nchunks, KVW), np.float16)
    kvs[:, :, 0:CHUNK] = kT.reshape(TILE, plan.nchunks, CHUNK)
    vem = kvs[:, :, CHUNK:CHUNK + 4 * VW].reshape(TILE, plan.nchunks, 4, VW)
    vem[:, :, :, 0:DIM] = vE.reshape(plan.nchunks, 4, TILE, DIM).transpose(2, 0, 1, 3)
    vem[:, :, :, DIM] = ind
    kvs[:, :, CHUNK + 4 * VW:KVW] = S
    kvs = np.ascontiguousarray(kvs.reshape(TILE, plan.nchunks * KVW))
    ST = np.ascontiguousarray(ST.reshape(32, plan.nchunks * TILE))

    qvalid = qslot2 >= 0
    qidx = np.where(qvalid, qslot2, 0)
    qT2 = np.zeros((DIM, plan.nsp2), np.float16)
    qT2[:, : plan.ng2] = np.where(qvalid[:, None], q_nodes[qidx], 0.0
                                  ).astype(np.float16).T
    return kvs, ST, qT2, qslot


# ---------------------------------------------------------------------------
# Device kernel emission
# ---------------------------------------------------------------------------

def _build_module(plan):
    import concourse.bacc as bacc
    import concourse.mybir as mybir
    import concourse.tile as tile
    from contextlib import ExitStack

    f16 = mybir.dt.float16
    bf = mybir.dt.bfloat16
    f32 = mybir.dt.float32
    NSP = plan.nsp
    NSP2 = plan.nsp2
    SL = plan.sl
    KVW = plan.kvw
    PR = HEADS * SL              # park rows per chunk (32)
    NR2 = plan.nr2               # total park rows
    NB2 = (NR2 + TILE - 1) // TILE   # 128-row blocks in final readback

    nc = bacc.Bacc("TRN2", debug=False, num_devices=NCORES)

    kvs_d = nc.dram_tensor("kvs", [TILE, plan.nchunks * KVW], f16,
                           kind="ExternalInput")
    ST_d = nc.dram_tensor("ST", [32, plan.nchunks * TILE], f16,
                          kind="ExternalInput")
    V4_d = nc.dram_tensor("V4", [32, 4 * JJ * HEADS], f16,
                          kind="ExternalInput")
    qT_d = nc.dram_tensor("qT", [DIM, NSP2], f16, kind="ExternalInput")
    Wq_d = nc.dram_tensor("Wq", [DIM, DIM], f16, kind="ExternalInput")
    WkH_d = nc.dram_tensor("WkH", [DIM, HEADS * DIM], f16, kind="ExternalInput")
    WVO_d = nc.dram_tensor("WVO", [DIM, HEADS * DIM], f16, kind="ExternalInput")
    ID_d = nc.dram_tensor("ID", [DIM, DIM], f32, kind="ExternalInput")
    bq_d = nc.dram_tensor("bq", [DIM, 1], f32, kind="ExternalInput")
    exb_d = nc.dram_tensor("exb", [DIM, 1], f32, kind="ExternalInput")
    bo_d = nc.dram_tensor("bo", [DIM, 1], f32, kind="ExternalInput")
    accD = nc.dram_tensor("accD", [NR2, CW], bf, kind="Internal")
    outT_d = nc.dram_tensor("outT", [DIM, NSP], f32, kind="ExternalOutput")

    Exp = mybir.ActivationFunctionType.Exp
    Ident = mybir.ActivationFunctionType.Identity
    mult = mybir.AluOpType.mult
    amax = mybir.AluOpType.max
    addop = mybir.AluOpType.add
    AxX = mybir.AxisListType.X

    with ExitStack() as ctx:
        tc = ctx.enter_context(tile.TileContext(nc))
        cp = ctx.enter_context(tc.tile_pool(name="const", bufs=1))
        sp = ctx.enter_context(tc.tile_pool(name="persist", bufs=1))
        iop = ctx.enter_context(tc.tile_pool(name="io", bufs=8))
        xp = ctx.enter_context(tc.tile_pool(name="work", bufs=6))
        pp = ctx.enter_context(tc.tile_pool(name="ps", bufs=2, space="PSUM"))

        def dmac(tile_ap, dram_ap):
            nc.sync.dma_start(out=tile_ap, in_=dram_ap)

        Wq_sb = cp.tile([DIM, DIM], f16); dmac(Wq_sb[:], Wq_d[:, :])
        WkH_sb = cp.tile([DIM, HEADS * DIM], f16); dmac(WkH_sb[:], WkH_d[:, :])
        WVO_sb = cp.tile([DIM, HEADS * DIM], f16); dmac(WVO_sb[:], WVO_d[:, :])
        ID_sb = cp.tile([DIM, DIM], f32); dmac(ID_sb[:], ID_d[:, :])
        V4_sb = cp.tile([32, 4 * JJ * HEADS], f16); dmac(V4_sb[:], V4_d[:, :])
        bq_sb = cp.tile([DIM, 1], f32); dmac(bq_sb[:], bq_d[:, :])
        exb_sb = cp.tile([DIM, 1], f32); dmac(exb_sb[:], exb_d[:, :])
        bo_sb = cp.tile([DIM, 1], f32); dmac(bo_sb[:], bo_d[:, :])
        qT_sb = sp.tile([DIM, NSP2], f16); dmac(qT_sb[:], qT_d[:, :])

        qp_sb = sp.tile([DIM, NSP2], f16)
        M_sb = sp.tile([DIM, 4 * NSP2], f16)

        # ---- Stage A: q projection + bias ----
        for b in range(NSP2 // CHUNK):
            sl = slice(b * CHUNK, (b + 1) * CHUNK)
            qp_ps = pp.tile([DIM, CHUNK], f32, tag="aux")
            nc.tensor.matmul(out=qp_ps[:], lhsT=Wq_sb[:], rhs=qT_sb[:, sl],
                             start=True, stop=True)
            nc.scalar.activation(out=qp_sb[:, sl], in_=qp_ps[:],
                                 func=Ident, bias=bq_sb[:, 0:1])

        # ---- Stage A: M matrices, (w,h)-interleaved, emitted JIT ----
        M_wh = M_sb[:].rearrange("p (w h) -> p h w", h=HEADS)
        NG = NSP2 // CHUNK

        def emit_mgroup(b):
            sl = slice(b * CHUNK, (b + 1) * CHUNK)
            for h in range(HEADS):
                M_ps = pp.tile([DIM, CHUNK], f32, tag="aux", name=f"M_ps{b}_{h}")
                nc.tensor.matmul(out=M_ps[:],
                                 lhsT=WkH_sb[:, h * DIM:(h + 1) * DIM],
                                 rhs=qp_sb[:, sl], start=True, stop=True)
                nc.scalar.copy(out=M_wh[:, h, sl], in_=M_ps[:])

        emit_mgroup(0)
        emit_mgroup(1)
        next_g = 2

        # ---- Steady state: chunk PAIRS; seg matmuls lag one pair behind ----
        npairs = (plan.nchunks + 1) // 2
        park = None
        pend = {}
        for pi2 in range(npairs + 1):
            while next_g < NG and pi2 >= 4 * next_g - 6:
                emit_mgroup(next_g)
                next_g += 1
            if pi2 < npairs:
                c0 = 2 * pi2
                pcs = [c for c in (c0, c0 + 1) if c < plan.nchunks]
                np_ = len(pcs)
                kv2 = iop.tile([TILE, 2 * KVW], f16, tag="kv")
                dmac(kv2[:, 0:np_ * KVW],
                     kvs_d[:, c0 * KVW:(c0 + np_) * KVW])
                st2 = iop.tile([32, 2 * TILE], f16, tag="st")
                dmac(st2[:, 0:np_ * TILE],
                     ST_d[:, c0 * TILE:(c0 + np_) * TILE])

                ex_ps = pp.tile([TILE, 2 * 4 * JJ * HEADS], f32, tag="score")
                for c2, ci in enumerate(pcs):
                    nc.tensor.matmul(
                        out=ex_ps[:, c2 * 64:(c2 + 1) * 64],
                        lhsT=st2[:, c2 * TILE:(c2 + 1) * TILE],
                        rhs=V4_sb[:],
                        start=True, stop=True)
                    for t in range(4):
                        m0 = 4 * ((ci * 4 + t) * JJ)
                        nc.tensor.matmul(
                            out=ex_ps[:, c2 * 64 + t * 16:c2 * 64 + t * 16 + 16],
                            lhsT=kv2[:, c2 * KVW + t * TILE:
                                     c2 * KVW + (t + 1) * TILE],
                            rhs=M_sb[:, m0:m0 + JJ * HEADS],
                            start=False, stop=True)

                exR_sb = xp.tile([TILE, 2 * 64], bf, tag="exR")
                nc.scalar.activation(out=exR_sb[:, 0:np_ * 64],
                                     in_=ex_ps[:, 0:np_ * 64], func=Exp,
                                     bias=exb_sb[:, 0:1])
                T_sb = xp.tile([TILE, 2 * 4 * HEADS * SL], f16, tag="T")
                for c2, ci in enumerate(pcs):
                    exE_sb = xp.tile([TILE, 4 * HEADS], f32, tag="exE")
                    nc.vector.tensor_reduce(
                        out=exE_sb[:].rearrange("p (t h) -> p t h", t=4),
                        in_=exR_sb[:, c2 * 64:(c2 + 1) * 64]
                            .rearrange("p (t q h) -> p t h q", t=4, q=JJ),
                        axis=AxX, op=addop)
                    S2v = kv2[:, c2 * KVW + CHUNK + 4 * VW:
                              c2 * KVW + CHUNK + 4 * VW + 4 * SL]
                    nc.vector.tensor_tensor(
                        out=T_sb[:, c2 * 4 * PR:(c2 + 1) * 4 * PR]
                            .rearrange("p (t h j) -> p t h j", t=4, h=HEADS),
                        in0=S2v.rearrange("p (t j) -> p t j", t=4)
                            .unsqueeze(2).to_broadcast([TILE, 4, HEADS, SL]),
                        in1=exE_sb[:].rearrange("p (t h) -> p t h", t=4)
                            .unsqueeze(-1).to_broadcast([TILE, 4, HEADS, SL]),
                        op=mult)
                pend[pi2] = (kv2, T_sb, pcs)

            if pi2 >= 1:
                kv2j, T_j, pcsj = pend.pop(pi2 - 1)
                for c2, cj in enumerate(pcsj):
                    gi = cj % GPC
                    if gi == 0:
                        park = pp.tile([PR, GPC * CW], f32, tag="park")
                    for t in range(4):
                        nc.tensor.matmul(
                            out=park[:, gi * CW:(gi + 1) * CW],
                            lhsT=T_j[:, c2 * 4 * PR + t * PR:
                                     c2 * 4 * PR + (t + 1) * PR],
                            rhs=kv2j[:, c2 * KVW + CHUNK + t * VW:
                                     c2 * KVW + CHUNK + t * VW + CW],
                            start=(t == 0), stop=(t == 3))
                    if gi == GPC - 1 or cj == plan.nchunks - 1:
                        g0 = (cj // GPC) * GPC
                        used = cj - g0 + 1
                        stage = xp.tile([PR, GPC * CW], bf, tag="stage")
                        nc.vector.tensor_copy(out=stage[:, 0:used * CW],
                                              in_=park[:, 0:used * CW])
                        nc.scalar.dma_start(
                            out=accD[g0 * PR:(cj + 1) * PR, :]
                                .rearrange("(c r) w -> r c w", r=PR),
                            in_=stage[:, 0:used * CW]
                                .rearrange("r (c w) -> r c w", w=CW))

        # ---- Final: readback, normalize, project with Wv@Wo per head ----
        # Two halves so half A's normalize/transpose overlaps half B's DMA.
        accR = sp.tile([TILE, NB2 * CW], bf)
        nc.gpsimd.memset(accR[:], 0.0)
        rden_sb = sp.tile([TILE, NB2], f32)
        nB_sb = sp.tile([TILE, NB2 * DIM], f32)
        nBT_sb = sp.tile([TILE, NB2 * TILE], f16)
        qs = [0, NB2 // 4, NB2 // 2, 3 * NB2 // 4, NB2]
        for b0, b1 in zip(qs, qs[1:]):
            full = min(b1 * TILE, NR2) // TILE - b0
            if full > 0:
                dmac(accR[:, b0 * CW:(b0 + full) * CW]
                     .rearrange("p (b w) -> p b w", w=CW),
                     accD[b0 * TILE:(b0 + full) * TILE, :]
                     .rearrange("(b p) w -> p b w", p=TILE))
            bf_ = b0 + full
            tail = NR2 - bf_ * TILE
            if 0 < tail < TILE and bf_ < b1:
                dmac(accR[0:tail, bf_ * CW:(bf_ + 1) * CW],
                     accD[bf_ * TILE:NR2, :])
            nc.vector.tensor_scalar(
                out=rden_sb[:, b0:b1].unsqueeze(-1),
                in0=accR[:, b0 * CW:b1 * CW]
                    .rearrange("p (b w) -> p b w", w=CW)[:, :, DIM:DIM + 1],
                scalar1=1e-30, scalar2=None, op0=amax)
            nc.vector.reciprocal(out=rden_sb[:, b0:b1], in_=rden_sb[:, b0:b1])
            nc.vector.tensor_tensor(
                out=nB_sb[:, b0 * DIM:b1 * DIM]
                    .rearrange("p (b w) -> p b w", w=DIM),
                in0=accR[:, b0 * CW:b1 * CW]
                    .rearrange("p (b w) -> p b w", w=CW)[:, :, 0:DIM],
                in1=rden_sb[:, b0:b1].unsqueeze(-1)
                    .to_broadcast([TILE, b1 - b0, DIM]),
                op=mult)
            for b in range(b0, b1):
                tp_ps = pp.tile([DIM, TILE], f32, tag="tp")
                nc.tensor.transpose(out=tp_ps[:],
                                    in_=nB_sb[:, b * DIM:(b + 1) * DIM],
                                    identity=ID_sb[:])
                nc.vector.tensor_copy(out=nBT_sb[:, b * TILE:(b + 1) * TILE],
                                      in_=tp_ps[:])

        # nBT cols are park rows: (ci, h, j).  For each ci-group and head,
        # a strided rhs selects that head's SL slot columns.
        nBT_r = nBT_sb[:, 0:NR2].rearrange("p (c h j) -> p c h j", h=HEADS, j=SL)
        CIG = CHUNK // SL        # chunks per output group (64)
        for gstart in range(0, plan.nchunks, CIG):
            gend = min(gstart + CIG, plan.nchunks)
            ncol = (gend - gstart) * SL
            out_ps = pp.tile([DIM, CHUNK], f32, tag="aux")
            for h in range(HEADS):
                nc.tensor.matmul(
                    out=out_ps[:, 0:ncol],
                    lhsT=WVO_sb[:, h * DIM:(h + 1) * DIM],
                    rhs=nBT_r[:, gstart:gend, h, :],
                    start=(h == 0), stop=(h == HEADS - 1))
            osb = xp.tile([DIM, CHUNK], f32, tag="osb")
            nc.scalar.activation(out=osb[:, 0:ncol], in_=out_ps[:, 0:ncol],
                                 func=Ident, bias=bo_sb[:, 0:1])
            dmac(outT_d[:, gstart * SL:gstart * SL + ncol], osb[:, 0:ncol])

    nc.compile()
    return nc


# ---------------------------------------------------------------------------
# Entry point
# ---------------------------------------------------------------------------

def _make_v4():
    BIG = 50.0
    V4 = np.zeros((32, 4 * JJ * HEADS), np.float16)
    for z in range(4 * JJ):
        V4[z, z * HEADS:(z + 1) * HEADS] = -BIG
    return V4


def _prepare(inputs):
    q_nodes = np.asarray(inputs["q_nodes"], np.float32)
    k_edges = np.asarray(inputs["k_edges"], np.float32)
    v_edges = np.asarray(inputs["v_edges"], np.float32)
    Wq = np.asarray(inputs["Wq"], np.float32)
    bq = np.asarray(inputs["bq"], np.float32)
    Wk = np.asarray(inputs["Wk"], np.float32)
    Wv = np.asarray(inputs["Wv"], np.float32)
    bv = np.asarray(inputs["bv"], np.float32)
    Wo = np.asarray(inputs["Wo"], np.float32)
    bo = np.asarray(inputs["bo"], np.float32)
    dst = np.asarray(inputs["edge_index"])[0].astype(np.int64)

    plan = _make_plan(dst)

    eorder = np.argsort(dst, kind="stable")
    starts = np.zeros(N + 1, np.int64)
    np.cumsum(np.bincount(dst, minlength=N), out=starts[1:])
    edges_of = [eorder[starts[n]: starts[n + 1]] for n in range(N)]

    # WVO[:, h-block] = Wv[:, 32h:32h+32] @ Wo[32h:32h+32, :]
    WVO = np.concatenate(
        [Wv[:, h * DH:(h + 1) * DH] @ Wo[h * DH:(h + 1) * DH, :]
         for h in range(HEADS)], axis=1)

    consts = {
        "Wq": Wq.astype(np.float16),
        "WkH": np.concatenate(
            [np.where((np.arange(DIM)[:, None] // DH == h),
                      (Wk * SCALE).T, 0.0) for h in range(HEADS)],
            axis=1).astype(np.float16),
        "WVO": WVO.astype(np.float16),
        "ID": np.eye(DIM, dtype=np.float32),
        "V4": _make_v4(),
        "bq": bq.reshape(DIM, 1).astype(np.float32),
        "exb": np.full((DIM, 1), EXB, np.float32),
        # sum(attn)==1 folds bv through Wo: out = (segv/den)@Wo + (bv@Wo + bo)
        "bo": (bv @ Wo + bo).reshape(DIM, 1).astype(np.float32),
    }
    return plan, dst, edges_of, consts, q_nodes, k_edges, v_edges, bo


def kernel(**inputs):
    from concourse.bass_utils import run_bass_kernel_spmd

    (plan, dst, edges_of, consts, q_nodes, k_edges, v_edges, bo) = _prepare(inputs)

    nc = _build_module(plan)

    in_maps = []
    slot_maps = []
    for c in range(NCORES):
        kvs, ST, qT2, qslot = _pack_core_inputs(plan, c, k_edges, v_edges,
                                                q_nodes, edges_of)
        m = {"kvs": kvs, "ST": ST, "qT": qT2}
        m.update(consts)
        in_maps.append(m)
        slot_maps.append(qslot)

    res = run_bass_kernel_spmd(nc, in_maps, core_ids=list(range(NCORES)))
    global LAST_RESULTS
    LAST_RESULTS = res

    out = np.zeros((N, DIM), np.float32)
    for c in range(NCORES):
        outT = res.results[c]["outT"]          # [DIM, nsp]
        qslot = slot_maps[c]
        valid = qslot >= 0
        out[qslot[valid]] = outT[:, : plan.nslot].T[valid]
    deg0 = plan.deg == 0
    if deg0.any():
        out[deg0] = bo
    return out
